# revision 9
# baseline (speedup 1.0000x reference)
"""Trainium2 Bass kernel for the NodePart segment-reduce problem.

Math (from the reference):
  phi_raw[n,c] = mean of the c-th 32-wide chunk of z[n,:]          [N, 8]
  phi        = softmax(phi_raw, axis=0)  (column-wise over nodes)
  r          = phi.sum(1);  node_weight = phi * (r.sum() - r)[:,None]
  mask       = (node_weight == rowmax(node_weight)) | (node_weight >= 1.0)
  x_parts[c] = where(mask[:,c,None], x, 0)

Key numerical facts exploited:
  * Output values are exact copies of x (or 0) — only the boolean mask is
    numerically sensitive, and it reduces to a per-node argmax over the 8
    columns of exp(phi_raw[n,c]) / S_c  (S_c = global column sum of exp).
  * phi_raw is bounded (|phi_raw| <~ 1), so the softmax max-subtraction is
    unnecessary for stability; skipping it changes results only at the
    ~1 ulp level (same level as any re-ordered float sum) and saves one
    all-reduce round.
  * r.sum() == 8 to within float rounding (column softmax sums to 1); the
    factor (8 - r[n]) is a positive per-node scalar, so it cannot change
    the per-node argmax.  It is still computed for the >= 1.0 threshold
    term (which never fires at these magnitudes, but is kept faithfully).

Sharding: nodes (dim 0) across 8 cores, 12500 rows each, zero/neg padded to
12544 = 14 batches * 7 tiles * 128 partitions.  One AllReduce(add) of the
8 per-column partial sums is the only collective.
"""

import numpy as np

import concourse.bass as bass
import concourse.bacc as bacc
import concourse.tile as tile
from concourse import mybir
from concourse.bass_utils import run_bass_kernel_spmd

NCORES = 8
C = 8            # communities
F = 256          # x feature dim
D = 256          # z feature dim
CHUNK = D // C   # 32
P = 128          # SBUF partitions
Z_PAD = -1.0e4   # exp(Z_PAD) == 0.0 in f32 -> padded rows add 0 to column sums

# results object of the last run (for test harnesses; unused by grading)
_last_results = None
_built_cache = {}


def _build(nsh: int, tb: int) -> bass.Bass:
    """Emit the SPMD program for one core holding `nsh` node rows."""
    nt = nsh // P            # node tiles of 128
    nb = nt // tb            # DMA batches of tb tiles
    assert nt * P == nsh and nb * tb == nt

    f32 = mybir.dt.float32
    u8 = mybir.dt.uint8
    AX = mybir.AxisListType.X
    EXP = mybir.ActivationFunctionType.Exp
    CPY = mybir.ActivationFunctionType.Copy
    alu = mybir.AluOpType

    nc = bacc.Bacc(
        "TRN2", target_bir_lowering=False, debug=False, num_devices=NCORES)
    x_d = nc.declare_dram_parameter("x", [nsh, F], f32, isOutput=False)
    z_d = nc.declare_dram_parameter("z", [nsh, D], f32, isOutput=False)
    xp_d = nc.declare_dram_parameter("xp", [C, nsh, F], f32, isOutput=True)
    mk_d = nc.declare_dram_parameter("mask", [nsh, C], u8, isOutput=True)

    with tile.TileContext(nc) as tc:
        with (
            tc.tile_pool(name="zp", bufs=2) as zp,
            tc.tile_pool(name="xq", bufs=2) as xq,
            tc.tile_pool(name="oq", bufs=2) as oq,
            tc.tile_pool(name="php", bufs=2) as php,
            tc.tile_pool(name="mq", bufs=2) as mq,
            tc.tile_pool(name="pp", bufs=1) as pp,
            tc.tile_pool(name="dr", bufs=1, space="DRAM") as dr,
        ):
            E = pp.tile([P, nt, C], f32)  # exp(phi_raw), node-tile major

            # ---- pass 1: z -> E = exp(chunk_sum / 32) ----
            for b in range(nb):
                zt = zp.tile([P, tb, D], f32)
                nc.sync.dma_start(
                    out=zt[:],
                    in_=z_d[b * tb * P:(b + 1) * tb * P, :].rearrange(
                        "(t p) f -> p t f", p=P),
                )
                ph = php.tile([P, tb, C, 1], f32)
                nc.vector.reduce_sum(
                    out=ph[:],
                    in_=zt[:].rearrange("p t (c k) -> p t c k", k=CHUNK),
                    axis=AX,
                )
                nc.scalar.activation(
                    out=E[:, b * tb:(b + 1) * tb, :],
                    in_=ph[:, :, :, 0],
                    func=EXP,
                    scale=1.0 / CHUNK,
                )

            # ---- local column sums over all nodes on this core ----
            sr = pp.tile([P, C, 1], f32)
            nc.vector.reduce_sum(
                out=sr[:], in_=E[:].rearrange("p t c -> p c t"), axis=AX)
            # compute engines cannot mix SBUF operands at different base
            # partitions: collapse the 128 per-partition partials via a tiny
            # SBUF->SBUF DMA into partition 0 + a free-dim reduce there.
            srow = pp.tile([1, P, C], f32)
            nc.sync.dma_start(out=srow[:], in_=sr[:, :, 0])
            s1 = pp.tile([1, C, 1], f32)
            nc.vector.reduce_sum(
                out=s1[:], in_=srow[:].rearrange("q a c -> q c a"), axis=AX)

            # ---- global column sums via AllReduce ----
            cin = dr.tile([1, C], f32)
            cout = dr.tile([1, C], f32, addr_space="Shared")
            nc.sync.dma_start(out=cin[:], in_=s1[0:1, :, 0])
            nc.gpsimd.collective_compute(
                "AllReduce",
                alu.add,
                replica_groups=[list(range(NCORES))],
                ins=[cin.opt()],
                outs=[cout.opt()],
            )
            sg = pp.tile([P, C], f32)
            nc.sync.dma_start(out=sg[:], in_=cout[:].broadcast_to((P, C)))
            sinvb = pp.tile([P, 1, C], f32)
            nc.vector.reciprocal(out=sinvb[:, 0, :], in_=sg[:])

            # ---- pass 2: mask + masked x writes ----
            for b in range(nb):
                lo, hi = b * tb * P, (b + 1) * tb * P
                xt = xq.tile([P, tb, F], f32)
                nc.sync.dma_start(
                    out=xt[:],
                    in_=x_d[lo:hi, :].rearrange("(t p) f -> p t f", p=P),
                )

                V = mq.tile([P, tb, C], f32)   # phi = E / S
                nc.vector.tensor_mul(
                    out=V[:],
                    in0=E[:, b * tb:(b + 1) * tb, :],
                    in1=sinvb[:].broadcast_to((P, tb, C)),
                )
                rmax = mq.tile([P, tb, 1], f32)
                nc.vector.reduce_max(out=rmax[:], in_=V[:], axis=AX)
                rsum = mq.tile([P, tb, 1], f32)
                nc.vector.reduce_sum(out=rsum[:], in_=V[:], axis=AX)
                sfac = mq.tile([P, tb, 1], f32)  # (8 - r)
                nc.scalar.activation(
                    out=sfac[:], in_=rsum[:], func=CPY, bias=float(C), scale=-1.0)
                W = mq.tile([P, tb, C], f32)   # node_weight
                nc.vector.tensor_mul(
                    out=W[:], in0=V[:], in1=sfac[:].broadcast_to((P, tb, C)))
                eq = mq.tile([P, tb, C], f32)
                nc.vector.tensor_tensor(
                    out=eq[:], in0=V[:], in1=rmax[:].broadcast_to((P, tb, C)),
                    op=alu.is_equal)
                ge = mq.tile([P, tb, C], f32)
                nc.vector.tensor_scalar(
                    out=ge[:], in0=W[:], scalar1=1.0, scalar2=None, op0=alu.is_ge)
                mf = mq.tile([P, tb, C], f32)
                nc.vector.tensor_max(out=mf[:], in0=eq[:], in1=ge[:])
                mu = mq.tile([P, tb, C], u8)
                nc.vector.tensor_copy(out=mu[:], in_=mf[:])
                nc.sync.dma_start(
                    out=mk_d[lo:hi, :].rearrange("(t p) c -> p t c", p=P),
                    in_=mu[:],
                )

                ot = oq.tile([P, C, tb, F], f32)
                for c in range(C):
                    for t in range(tb):
                        if (c * tb + t) % 2 == 0:
                            nc.vector.tensor_scalar_mul(
                                out=ot[:, c, t, :], in0=xt[:, t, :],
                                scalar1=mf[:, t, c:c + 1])
                        else:
                            nc.scalar.mul(
                                out=ot[:, c, t, :], in_=xt[:, t, :],
                                mul=mf[:, t, c:c + 1])
                for c in range(C):
                    nc.sync.dma_start(
                        out=xp_d[c, lo:hi, :].rearrange("(t p) f -> p t f", p=P),
                        in_=ot[:, c, :, :],
                    )
    nc.compile()
    return nc


def _get_program(nsh: int, tb: int) -> bass.Bass:
    key = (nsh, tb)
    if key not in _built_cache:
        _built_cache[key] = _build(nsh, tb)
    return _built_cache[key]


def _shard_inputs(x: np.ndarray, z: np.ndarray, nsh_raw: int, nsh: int):
    in_maps = []
    for k in range(NCORES):
        lo = k * nsh_raw
        xs = np.zeros((nsh, F), np.float32)
        zs = np.full((nsh, D), Z_PAD, np.float32)
        xs[:nsh_raw] = x[lo:lo + nsh_raw]
        zs[:nsh_raw] = z[lo:lo + nsh_raw]
        in_maps.append({"x": xs, "z": zs})
    return in_maps


def kernel(x: np.ndarray, z: np.ndarray):
    global _last_results
    x = np.ascontiguousarray(x, dtype=np.float32)
    z = np.ascontiguousarray(z, dtype=np.float32)
    n = x.shape[0]
    assert n % NCORES == 0
    nsh_raw = n // NCORES                     # 12500
    tb = 7
    step = tb * P                             # 896
    nsh = ((nsh_raw + step - 1) // step) * step  # 12544

    prog = _get_program(nsh, tb)
    in_maps = _shard_inputs(x, z, nsh_raw, nsh)
    try:
        res = run_bass_kernel_spmd(prog, in_maps, list(range(NCORES)))
    except ModuleNotFoundError:
        # BASS_TRACE set but the NTFF profile hook isn't available in this
        # container: retry with tracing disabled.
        import os
        os.environ["BASS_NEVER_TRACE"] = "1"
        res = run_bass_kernel_spmd(prog, in_maps, list(range(NCORES)))
    _last_results = res
    outs = res.results

    xp = np.empty((C, n, F), np.float32)
    mask = np.empty((n, C), np.bool_)
    for k in range(NCORES):
        lo = k * nsh_raw
        xp[:, lo:lo + nsh_raw, :] = outs[k]["xp"][:, :nsh_raw, :]
        mask[lo:lo + nsh_raw, :] = outs[k]["mask"][:nsh_raw, :] != 0
    return xp, mask
